# revision 1
# baseline (speedup 1.0000x reference)
"""BitLinear forward kernel for Trainium2 (8 NeuronCores) — v8.

v4 = v3 (sharded W-quant + fp8 code AllGather) with the main matmul
restructured to amortize PE weight loads: the gathered ternary code
block codeT[:, c, dp, :, j*128:(j+1)*128] is the STATIONARY operand and
sign(x)^T streams 1024 token-columns per instruction (4:1 stream:load),
accumulating y^T [128 o, 1024 t] in 2-bank psum tiles. With o on psum
partitions, the per-row alpha becomes a per-partition activation scale,
so y^T eviction runs on ACT (Copy * alpha_m) with no alpha broadcast;
y is written transposed [O, T] per core and untransposed on the host.
"""

import sys

for _p in ("/opt/trn_rl_repo", "/opt/trn_rl_repo/concourse"):
    if _p not in sys.path:
        sys.path.insert(0, _p)

import numpy as np

import concourse.bass as bass
import concourse.tile as tile
import concourse.mybir as mybir
from concourse import bacc
from concourse.bass_utils import run_bass_kernel_spmd
from concourse.masks import make_identity

# Problem shape (hardcoded per contract)
B, S, D, O = 4, 4096, 2048, 2048
N_CORES = 8
T = (B * S) // N_CORES  # 2048 token rows per core
OSH = O // N_CORES      # 256 W rows quantized per core
DELTA_W = 0.05

P = 128
NT = T // P     # 16 x row-blocks
NWL = OSH // P  # 2 local W blocks
ND = D // P     # 16 d-tiles
DP = ND // 2    # 8 DoubleRow d-pairs
NOB = O // P    # 16 output o-blocks
TH = T // 2     # 1024 streamed t-columns per matmul
JW = OSH        # 256 o-columns per gathered chunk

F32 = mybir.dt.float32
BF16 = mybir.dt.bfloat16
FP8 = mybir.dt.float8e4
U16 = mybir.dt.uint16

Alu = mybir.AluOpType
Act = mybir.ActivationFunctionType

_CACHE = {}


def _build(with_bias: bool):
    nc = bacc.Bacc("TRN2", target_bir_lowering=False, debug=False,
                   num_devices=N_CORES)
    x_d = nc.dram_tensor("x", [T, D], F32, kind="ExternalInput").ap()
    w_d = nc.dram_tensor("W", [OSH, D], F32, kind="ExternalInput").ap()
    # y is stored transposed [O, T]; host transposes back
    y_d = nc.dram_tensor("y", [O, T], BF16, kind="ExternalOutput").ap()
    if with_bias:
        b_d = nc.dram_tensor("b", [O], F32, kind="ExternalInput").ap()

    # collective bounce buffers, one per local W block (half-shards);
    # payload = 2048 B of transposed code + 4 B of alpha per partition
    CCW = DP * 2 * P + 4  # 2052
    cc_in = [nc.dram_tensor(f"cc_in{k}", [P, CCW], FP8).ap()
             for k in range(NWL)]
    cc_out = [nc.dram_tensor(f"cc_out{k}", [N_CORES, P, CCW], FP8,
                             addr_space="Shared").ap()
              for k in range(NWL)]

    groups = [list(range(N_CORES))]

    with tile.TileContext(nc) as tc:
        with (
            tc.tile_pool(name="const", bufs=1) as const,
            tc.tile_pool(name="big", bufs=1) as big,
            tc.tile_pool(name="stats", bufs=1) as stats,
            tc.tile_pool(name="wload", bufs=2) as wload,
            tc.tile_pool(name="awc", bufs=2) as awc_pool,
            tc.tile_pool(name="cpos", bufs=2) as cpos_pool,
            tc.tile_pool(name="codem", bufs=2) as codem_pool,
            tc.tile_pool(name="xload", bufs=3) as xload,
            tc.tile_pool(name="xsign", bufs=2) as xsign,
            tc.tile_pool(name="junk", bufs=1) as junk_pool,
            tc.tile_pool(name="ystrip", bufs=3) as ystrip_pool,
            tc.tile_pool(name="psum_mm", bufs=6, space="PSUM") as psum_mm,
            tc.tile_pool(name="psum_tx", bufs=2, space="PSUM") as psum_tx,
        ):
            id_bf = const.tile([P, P], BF16, tag="id_bf")
            make_identity(nc, id_bf[:])

            # fp8 operand tensors
            xT = big.tile([P, DP, 2, T], FP8, tag="xT")
            codeT_sh = big.tile([P, NWL, DP, 2, P], FP8, tag="codeT_sh")
            codeT = big.tile([P, N_CORES, NWL, DP, 2, P], FP8, tag="codeT")
            alpha_sb = big.tile([P, NOB], F32, tag="alpha_sb")
            if with_bias:
                bias_sb = big.tile([P, NOB], F32, tag="bias_sb")
                nc.scalar.dma_start(
                    bias_sb[:], b_d.rearrange("(w p) -> p w", p=P)[:, :])

            # per-row stats, one column per local W block
            S_all = stats.tile([P, NWL], F32, tag="S")
            negmean = stats.tile([P, NWL], F32, tag="negmean")
            T_all = stats.tile([P, NWL], F32, tag="T")
            thr = stats.tile([P, NWL], F32, tag="thr")
            negthr = stats.tile([P, NWL], F32, tag="negthr")
            hi = stats.tile([P, NWL], F32, tag="hi")
            lo = stats.tile([P, NWL], F32, tag="lo")
            npos = stats.tile([P, NWL], F32, tag="npos")
            cmacc = stats.tile([P, NWL], F32, tag="cmacc")
            R_all = stats.tile([P, NWL], F32, tag="R")
            den = stats.tile([P, NWL], F32, tag="den")
            num = stats.tile([P, NWL], F32, tag="num")
            denc = stats.tile([P, NWL], F32, tag="denc")
            rden = stats.tile([P, NWL], F32, tag="rden")
            alpha_m = stats.tile([P, NWL], F32, tag="alpha_m")

            junk = junk_pool.tile([P, D], BF16, tag="junk")

            def w_block(k):
                ks = slice(k, k + 1)
                wt = wload.tile([P, D], F32)
                nc.scalar.dma_start(wt[:], w_d[k * P:(k + 1) * P, :])
                # S = row sum (ACT Copy with accumulate, junk output)
                nc.scalar.activation(
                    out=junk[:], in_=wt[:], func=Act.Copy,
                    accum_out=S_all[:, ks])
                nc.scalar.mul(negmean[:, ks], S_all[:, ks], -1.0 / D)
                aWc = awc_pool.tile([P, D], F32)
                nc.scalar.activation(
                    out=aWc[:], in_=wt[:], func=Act.Abs,
                    bias=negmean[:, ks], accum_out=T_all[:, ks])
                nc.scalar.mul(thr[:, ks], T_all[:, ks], DELTA_W / D)
                nc.scalar.mul(negthr[:, ks], T_all[:, ks], -DELTA_W / D)
                nc.vector.tensor_sub(hi[:, ks], thr[:, ks], negmean[:, ks])
                nc.vector.tensor_sub(lo[:, ks], negthr[:, ks], negmean[:, ks])
                nc.scalar.activation(
                    out=junk[:], in_=aWc[:], func=Act.Relu,
                    bias=negthr[:, ks], accum_out=R_all[:, ks])
                cp = cpos_pool.tile([P, D], BF16)
                nc.vector.tensor_scalar(
                    out=cp[:], in0=wt[:], scalar1=hi[:, ks], scalar2=0.0,
                    op0=Alu.is_ge, op1=Alu.add, accum_out=npos[:, ks])
                cm = codem_pool.tile([P, D], BF16)
                nc.vector.scalar_tensor_tensor(
                    out=cm[:], in0=wt[:], scalar=lo[:, ks], in1=cp[:],
                    op0=Alu.is_le, op1=Alu.subtract,
                    accum_out=cmacc[:, ks])
                # transpose the 16 d-tiles of code_m through the PE (bf16)
                for g in range(2):
                    ps = psum_tx.tile([P, 8 * P], BF16)
                    for j in range(8):
                        di = 8 * g + j
                        nc.tensor.matmul(
                            ps[:, j * P:(j + 1) * P],
                            cm[:, di * P:(di + 1) * P],
                            id_bf[:], is_transpose=True)
                    nc.vector.tensor_copy(
                        out=codeT_sh[:, k, 4 * g:4 * g + 4, :, :],
                        in_=ps.rearrange("p (a b t) -> p a b t",
                                         a=4, b=2, t=P)[:, :, :, :])

            def alpha_local(k):
                qs = slice(k, k + 1)
                nc.vector.scalar_tensor_tensor(
                    out=den[:, qs], in0=npos[:, qs], scalar=2.0,
                    in1=cmacc[:, qs], op0=Alu.mult, op1=Alu.add)
                nc.vector.tensor_mul(num[:, qs], thr[:, qs], den[:, qs])
                nc.vector.tensor_add(num[:, qs], num[:, qs], R_all[:, qs])
                nc.vector.tensor_scalar_max(denc[:, qs], den[:, qs], 1.0)
                nc.vector.reciprocal(rden[:, qs], denc[:, qs])
                nc.vector.scalar_tensor_tensor(
                    out=alpha_m[:, qs], in0=num[:, qs], scalar=-1.0,
                    in1=rden[:, qs], op0=Alu.mult, op1=Alu.mult)

            def cc_send(k):
                nc.sync.dma_start(
                    cc_in[k][:, 0:DP * 2 * P].rearrange(
                        "p (a b j) -> p a b j", a=DP, b=2, j=P)[:, :, :, :],
                    codeT_sh[:, k, :, :, :])
                nc.sync.dma_start(
                    cc_in[k][:, DP * 2 * P:].bitcast(F32),
                    alpha_m[:, k:k + 1])
                nc.gpsimd.collective_compute(
                    "AllGather", Alu.bypass, replica_groups=groups,
                    ins=[cc_in[k][:, :].opt()],
                    outs=[cc_out[k][:, :, :].opt()])

            def cc_recv(k):
                # per source core: code chunk + alpha column
                for c in range(N_CORES):
                    ob = c * NWL + k
                    nc.sync.dma_start(
                        codeT[:, c, k, :, :, :],
                        cc_out[k][c, :, 0:DP * 2 * P].rearrange(
                            "p (a b j) -> p a b j",
                            a=DP, b=2, j=P)[:, :, :, :])
                    nc.sync.dma_start(
                        alpha_sb[:, ob:ob + 1],
                        cc_out[k][c, :, DP * 2 * P:].bitcast(F32))

            def x_block(ti):
                xb = xload.tile([P, D], BF16)
                nc.gpsimd.dma_start(xb[:], x_d[ti * P:(ti + 1) * P, :])
                xq = xsign.tile([P, D], BF16)
                nc.vector.tensor_scalar(
                    out=xq.bitcast(U16)[:], in0=xb.bitcast(U16)[:],
                    scalar1=0x8000, scalar2=0x3F80,
                    op0=Alu.bitwise_and, op1=Alu.bitwise_or)
                for g in range(2):
                    ps = psum_tx.tile([P, 8 * P], BF16)
                    for j in range(8):
                        di = 8 * g + j
                        nc.tensor.matmul(
                            ps[:, j * P:(j + 1) * P],
                            xq[:, di * P:(di + 1) * P],
                            id_bf[:], is_transpose=True)
                    # split evictions across DVE and ACT so neither
                    # engine paces the x phase alone
                    src_ap = ps.rearrange("p (a b t) -> p a b t",
                                          a=4, b=2, t=P)[:, :, :, :]
                    dst_ap = xT[:, 4 * g:4 * g + 4, :, ti * P:(ti + 1) * P]
                    if g == 0:
                        nc.vector.tensor_copy(out=dst_ap, in_=src_ap)
                    else:
                        nc.scalar.activation(out=dst_ap, in_=src_ap,
                                             func=Act.Copy)

            def mm_block(ob):
                """y^T[ob*128:(ob+1)*128, :] : stationary code, stream x.

                dp outer / t-quarter inner: 4 consecutive matmuls share
                the stationary, into 4 one-bank psum tiles.
                """
                c, jb = divmod(ob, NWL)  # o = c*256 + jb*128 + p
                ysT = ystrip_pool.tile([P, T], BF16)
                pss = [psum_mm.tile([P, T // 4], F32, tag="mmps",
                                    name=f"mmps{tq}")
                       for tq in range(4)]
                for dp in range(DP):
                    for tq in range(4):
                        nc.tensor.matmul(
                            pss[tq][:],
                            codeT[:, c, jb, dp, :, :],
                            xT[:, dp, :, tq * (T // 4):(tq + 1) * (T // 4)],
                            start=(dp == 0), stop=(dp == DP - 1),
                            perf_mode=mybir.MatmulPerfMode.DoubleRow)
                # y^T = psum * alpha_m  (per-partition scale, on ACT)
                for tq in range(4):
                    nc.scalar.activation(
                        out=ysT[:, tq * (T // 4):(tq + 1) * (T // 4)],
                        in_=pss[tq][:],
                        func=Act.Copy, scale=alpha_sb[:, ob:ob + 1])
                if with_bias:
                    nc.vector.tensor_scalar(
                        out=ysT[:], in0=ysT[:],
                        scalar1=bias_sb[:, ob:ob + 1], scalar2=None,
                        op0=Alu.add)
                nc.sync.dma_start(y_d[ob * P:(ob + 1) * P, :], ysT[:])

            # ---- emission ----
            w_block(0)
            alpha_local(0)
            cc_send(0)
            w_block(1)
            alpha_local(1)
            cc_send(1)
            for ti in range(NT):
                x_block(ti)
            for k in range(NWL):
                cc_recv(k)
                for c in range(N_CORES):
                    mm_block(c * NWL + k)

    nc.compile()
    return nc


def _get_nc(with_bias: bool):
    key = with_bias
    if key not in _CACHE:
        _CACHE[key] = _build(with_bias)
    return _CACHE[key]


def kernel(x: np.ndarray, W: np.ndarray, b: np.ndarray) -> np.ndarray:
    x = np.asarray(x, dtype=np.float32)
    W = np.ascontiguousarray(W, dtype=np.float32)
    b = np.asarray(b, dtype=np.float32)
    with_bias = bool(np.any(b))

    nc = _get_nc(with_bias)

    xf = np.ascontiguousarray(x.reshape(B * S, D))
    in_maps = []
    for c in range(N_CORES):
        m = {"x": np.ascontiguousarray(xf[c * T:(c + 1) * T]),
             "W": np.ascontiguousarray(W[c * OSH:(c + 1) * OSH])}
        if with_bias:
            m["b"] = b
        in_maps.append(m)

    res = run_bass_kernel_spmd(nc, in_maps, core_ids=list(range(N_CORES)))
    # per-core y is [O, T]; transpose back and stack along tokens
    y = np.concatenate(
        [np.asarray(res.results[c]["y"]).astype(np.float32).T
         for c in range(N_CORES)], axis=0)
    return np.ascontiguousarray(y.reshape(B, S, O))


if __name__ == "__main__":
    rng = np.random.default_rng(0)
    x = rng.standard_normal((B, S, D), dtype=np.float32)
    W = rng.standard_normal((O, D), dtype=np.float32) * 0.03
    b = np.zeros((O,), dtype=np.float32)
    y = kernel(x, W, b)
    print("kernel ran, y shape", y.shape, "mean|y|", np.abs(y).mean())

